# revision 1
# baseline (speedup 1.0000x reference)
"""Trainium2 Bass kernel for nn_AttnBlock (dense transformer block).

Strategy (pure data-parallel over batch, 8 cores):
  - Each core gets B/8 = 512 samples; all weights replicated.
  - Algebraic fusion (host-side, weights only):
      * attention applied to raw x:  y_h = attn_h @ x    (per sample)
      * V-projection and output projection fused: Wvp_h = Wv_h @ Wp_h
      * per-head bias term folded in as extra contraction rows using the
        gate vector:  proj += gate[b,:] @ (bv_h @ Wp_h)
      * softmax normalization (1/Z) and gate applied as one row-scale
        between the attention matmul and the fused projection.
  - All dense GEMMs run in "T-layout" (features on partitions, rows on
    free dim) with weights as the stationary operand; bf16 operands with
    fp32 PSUM accumulation.
  - Per-sample attention matmuls are batched 12 samples at a time using
    block-diagonal [120,120] attention tiles so the PE runs dense work.

Self-contained: hardcodes shapes; imports only the concourse stack.
"""

import math
import os
import sys

import numpy as np

for _p in ("/opt/trn_rl_repo", os.path.expanduser("~/.axon_site/_ro/trn_rl_repo")):
    if os.path.isdir(_p) and _p not in sys.path:
        sys.path.insert(0, _p)

import ml_dtypes  # noqa: E402

import concourse.bass as bass  # noqa: E402
import concourse.mybir as mybir  # noqa: E402
import concourse.tile as tile  # noqa: E402
from concourse import bacc  # noqa: E402
from concourse.masks import make_identity  # noqa: E402

F32 = mybir.dt.float32
BF16 = mybir.dt.bfloat16
F32R = mybir.dt.float32r
AF = mybir.ActivationFunctionType
ALU = mybir.AluOpType

# Problem shapes (hardcoded per spec)
B, S, F, D, H = 4096, 10, 512, 512, 4
EPS = 1e-5
NCORES = 8
BC = B // NCORES          # samples per core = 512
P = 128

# Tiling
C = 32                    # samples per chunk
NCH = BC // C             # 16 chunks
MC = C * S                # 320 rows per chunk
WINS = (12, 12, 8)        # samples per attention window (sum = C)
FT = F // P               # 4 input-feature tiles
TT = (H * D) // P         # 16 q/k output tiles
D1T = (4 * D) // P        # 16 ffn hidden tiles
DPT = D // P              # 4 d_model tiles


def build_kernel(apply_ln_affine: bool, nch: int = NCH, debug: bool = False):
    MR = nch * MC  # rows handled by this program
    nc = bacc.Bacc(None, target_bir_lowering=False, debug=debug)
    names = {}

    _lp = nc.allow_low_precision(reason="float32r intermediates are 4-byte")
    _lp.__enter__()
    with tile.TileContext(nc) as tc:
        with tc.tile_pool(name="dram", bufs=1, space="DRAM") as dram:
            # per-core inputs (bf16 x, prepared on host)
            x_bf = dram.tile([MR, F], BF16, kind="ExternalInput", name="x_bf", uniquify=False)
            # prepared weights (host-fused / pre-scaled), all bf16
            wq_d = dram.tile([F, H * D], BF16, kind="ExternalInput", name="wq_p", uniquify=False)
            wk_d = dram.tile([F, H * D], BF16, kind="ExternalInput", name="wk_p", uniquify=False)
            wvp_d = dram.tile([H * F, D], BF16, kind="ExternalInput", name="wvp_p", uniquify=False)
            w1_d = dram.tile([D, 4 * D], BF16, kind="ExternalInput", name="w1_p", uniquify=False)
            w2_d = dram.tile([4 * D, D], BF16, kind="ExternalInput", name="w2_p", uniquify=False)
            wg_d = dram.tile([F, H], BF16, kind="ExternalInput", name="wg_p", uniquify=False)
            cg_d = dram.tile([H, D], BF16, kind="ExternalInput", name="cg_p", uniquify=False)
            bqc_d = dram.tile([P, TT], F32, kind="ExternalInput", name="bqc_p", uniquify=False)
            bkc_d = dram.tile([P, TT], F32, kind="ExternalInput", name="bkc_p", uniquify=False)
            bpc_d = dram.tile([P, DPT], F32, kind="ExternalInput", name="bpc_p", uniquify=False)
            b1c_d = dram.tile([P, D1T], F32, kind="ExternalInput", name="b1c_p", uniquify=False)
            b2c_d = dram.tile([P, DPT], F32, kind="ExternalInput", name="b2c_p", uniquify=False)
            bg_d = dram.tile([1, H], BF16, kind="ExternalInput", name="bg_p", uniquify=False)
            mask_d = dram.tile([120, 120], BF16, kind="ExternalInput", name="mask_p", uniquify=False)
            if apply_ln_affine:
                ln_d = dram.tile([4, D], F32, kind="ExternalInput", name="ln_p", uniquify=False)
            out_d = dram.tile([MR, F], F32, kind="ExternalOutput", name="out", uniquify=False)
        names["out"] = "out"

        from contextlib import ExitStack
        _stack = ExitStack()
        const = _stack.enter_context(tc.tile_pool(name="const", bufs=1))
        wts = _stack.enter_context(tc.tile_pool(name="wts", bufs=1))
        act = _stack.enter_context(tc.tile_pool(name="act", bufs=1))
        f32w = _stack.enter_context(tc.tile_pool(name="f32w", bufs=1))
        psq = _stack.enter_context(tc.tile_pool(name="psq", bufs=2, space="PSUM"))
        psb = _stack.enter_context(tc.tile_pool(name="psb", bufs=4, space="PSUM"))
        psr = _stack.enter_context(tc.tile_pool(name="psr", bufs=2, space="PSUM"))

        # ---- constants ----
        ident = const.tile([P, P], F32, tag="ident")
        make_identity(nc, ident)
        ones_row_bf = const.tile([1, 512], BF16, tag="ones_row_bf")
        nc.vector.memset(ones_row_bf[:], 1.0)
        ones_tmp = const.tile([P, P], F32, tag="ones_tmp")
        nc.vector.memset(ones_tmp[:], 1.0)
        ones_row_f32 = const.tile([1, P], F32R, tag="ones_row_f32")
        nc.vector.tensor_copy(ones_row_f32[:], ones_tmp[0:1, :])
        ones_col_f32 = const.tile([P, 1], F32R, tag="ones_col_f32")
        nc.vector.tensor_copy(ones_col_f32[:], ones_tmp[:, 0:1])
        ones_col_bf = const.tile([P, 1], BF16, tag="ones_col_bf")
        nc.vector.memset(ones_col_bf[:], 1.0)
        eps_sb = const.tile([1, 1], F32, tag="eps")
        nc.vector.memset(eps_sb[:], EPS)
        mask_bd = const.tile([120, 120], BF16, tag="mask_bd")
        nc.gpsimd.dma_start(mask_bd[:], mask_d[:])

        # ---- resident weights ----
        wq_sb = wts.tile([P, FT, H * D], BF16, tag="wq")
        wk_sb = wts.tile([P, FT, H * D], BF16, tag="wk")
        wvp_sb = wts.tile([P, TT, D], BF16, tag="wvp")
        w1_sb = wts.tile([P, FT, 4 * D], BF16, tag="w1")
        w2_sb = wts.tile([P, D1T, D], BF16, tag="w2")
        wg_sb = wts.tile([P, FT, H], BF16, tag="wg")
        cg_sb = wts.tile([H, D], BF16, tag="cg")
        bqc_sb = wts.tile([P, TT], F32, tag="bqc")
        bkc_sb = wts.tile([P, TT], F32, tag="bkc")
        bpc_sb = wts.tile([P, DPT], F32, tag="bpc")
        b1c_sb = wts.tile([P, D1T], F32, tag="b1c")
        b2c_sb = wts.tile([P, DPT], F32, tag="b2c")
        bg_sb = wts.tile([1, H], BF16, tag="bg")
        # small tensors first so early chunks aren't blocked behind big DMAs
        nc.sync.dma_start(bqc_sb[:], bqc_d[:])
        nc.sync.dma_start(bkc_sb[:], bkc_d[:])
        nc.sync.dma_start(bpc_sb[:], bpc_d[:])
        nc.sync.dma_start(b1c_sb[:], b1c_d[:])
        nc.sync.dma_start(b2c_sb[:], b2c_d[:])
        nc.sync.dma_start(bg_sb[:], bg_d[:])
        nc.sync.dma_start(wg_sb[:], wg_d[:].rearrange("(t p) n -> p t n", p=P))
        nc.sync.dma_start(cg_sb[:], cg_d[:])
        nc.sync.dma_start(wq_sb[:], wq_d[:].rearrange("(t p) n -> p t n", p=P))
        nc.sync.dma_start(wk_sb[:], wk_d[:].rearrange("(t p) n -> p t n", p=P))
        nc.sync.dma_start(wvp_sb[:], wvp_d[:].rearrange("(t p) n -> p t n", p=P))
        nc.sync.dma_start(w1_sb[:], w1_d[:].rearrange("(t p) n -> p t n", p=P))
        nc.sync.dma_start(w2_sb[:], w2_d[:].rearrange("(t p) n -> p t n", p=P))
        if apply_ln_affine:
            ln_sb = wts.tile([P, 4, DPT], F32, tag="ln")
            # ln_d rows: g1, be1, g2, be2 ; [512] -> [p, dpt]
            nc.sync.dma_start(
                ln_sb[:], ln_d[:].rearrange("r (t p) -> p r t", p=P)
            )

        def evac_engine(i):
            return nc.vector if (i % 2 == 0) else nc.scalar

        def copy_out(eng, dst, src):
            if eng is nc.vector:
                nc.vector.tensor_copy(dst, src)
            else:
                nc.scalar.copy(dst, src)

        x_flat = x_bf[:]  # [5120, 512] bf16 dram
        out_flat = out_d[:]

        for ch in range(nch):
            m0 = ch * MC

            # ---- load x: T-layout via DMA transpose; windows in row layout
            xt = act.tile([P, FT, MC], BF16, tag="xt", bufs=2)
            for ft in range(FT):
                nc.scalar.dma_start(
                    xt[:, ft, :],
                    x_flat[m0:m0 + MC, ft * P:(ft + 1) * P],
                    transpose=True,
                )
            xw = act.tile([P, len(WINS), F], BF16, tag="xw", bufs=2)
            wo = 0
            for w, wn in enumerate(WINS):
                nc.gpsimd.dma_start(
                    xw[:wn * S, w, :], x_flat[m0 + wo * S:m0 + (wo + wn) * S, :]
                )
                wo += wn

            # ---- Q/K projections (T-layout out, bf16) ----
            qt = act.tile([P, TT, MC], BF16, tag="qt")
            kt = act.tile([P, TT, MC], BF16, tag="kt")
            for which, (wsb, bsb, dst) in enumerate(
                ((wq_sb, bqc_sb, qt), (wk_sb, bkc_sb, kt))
            ):
                for t in range(TT):
                    ps = psq.tile([P, 512], F32, tag="qk")
                    for ft in range(FT):
                        nc.tensor.matmul(
                            ps[:, :MC],
                            lhsT=wsb[:, ft, t * P:(t + 1) * P],
                            rhs=xt[:, ft, :],
                            start=(ft == 0),
                            stop=(ft == FT - 1),
                        )
                    nc.vector.tensor_scalar_add(
                        dst[:, t, :], ps[:, :MC], bsb[:, t:t + 1]
                    )

            # ---- gate: softmax(x.mean(1) @ Wg + bg) ----
            xm_bf = act.tile([P, FT, C], BF16, tag="xm", bufs=2)
            for ft in range(FT):
                xm = f32w.tile([P, C], F32, tag="xmf", bufs=2)
                nc.vector.tensor_reduce(
                    xm[:, :],
                    xt[:, ft, :].rearrange("p (b s) -> p b s", s=S),
                    axis=mybir.AxisListType.X,
                    op=ALU.add,
                )
                nc.vector.tensor_copy(xm_bf[:, ft, :], xm[:, :])
            psg = psr.tile([C, H], F32, tag="rows")
            for ft in range(FT):
                nc.tensor.matmul(
                    psg[:, :],
                    lhsT=xm_bf[:, ft, :],
                    rhs=wg_sb[:, ft, :],
                    start=(ft == 0),
                    stop=False,
                )
            nc.tensor.matmul(
                psg[:, :],
                lhsT=ones_row_bf[0:1, :C],
                rhs=bg_sb[0:1, :],
                start=False,
                stop=True,
            )
            eg = f32w.tile([C, H], F32, tag="eg", bufs=2)
            zg = f32w.tile([C, 1], F32, tag="zg", bufs=2)
            nc.scalar.activation(eg[:, :], psg[:, :], AF.Exp, accum_out=zg[:, :])
            rzg = f32w.tile([C, 1], F32, tag="rzg", bufs=2)
            nc.vector.reciprocal(rzg[:, :], zg[:, :])
            gatef = f32w.tile([C, H], F32, tag="gatef", bufs=2)
            nc.vector.tensor_scalar_mul(gatef[:, :], eg[:, :], rzg[:, :])
            # transpose gate -> [H, C] then replicate over s -> [H, MC] bf16
            psgt = psr.tile([H, C], F32, tag="rows")
            nc.tensor.transpose(psgt[:, :], gatef[:, :], ident[:C, :C])
            gft = f32w.tile([H, C], F32, tag="gft", bufs=2)
            nc.vector.tensor_copy(gft[:, :], psgt[:, :])
            grep = act.tile([H, MC], BF16, tag="grep", bufs=2)
            for s in range(S):
                nc.vector.tensor_copy(
                    grep[:, :].rearrange("h (b s) -> h b s", s=S)[:, :, s], gft[:, :]
                )

            # ---- attention windows ----
            ysc = act.tile([P, TT, MC], BF16, tag="ysc")
            wo = 0
            for w, wn in enumerate(WINS):
                L = wn * S
                psz = psr.tile([1, 512], F32, tag="rows")
                atts = []
                for h in range(H):
                    pss = psq.tile([P, 512], F32, tag="qk")
                    for dt in range(FT):
                        t = h * FT + dt
                        nc.tensor.matmul(
                            pss[:L, :L],
                            lhsT=kt[:, t, wo * S:wo * S + L],
                            rhs=qt[:, t, wo * S:wo * S + L],
                            start=(dt == 0),
                            stop=(dt == FT - 1),
                        )
                    es = act.tile([120, 128], BF16, tag="es", bufs=6)
                    nc.scalar.activation(es[:L, :L], pss[:L, :L], AF.Exp)
                    abd = act.tile([120, 128], BF16, tag="abd", bufs=8)
                    nc.vector.tensor_mul(abd[:L, :L], es[:L, :L], mask_bd[:L, :L])
                    nc.tensor.matmul(
                        psz[0:1, h * L:h * L + L],
                        lhsT=ones_col_bf[:L, 0:1],
                        rhs=abd[:L, :L],
                        start=True,
                        stop=True,
                    )
                    atts.append(abd)
                # w_row = gate/(Z) in (h, bw, s) order
                grow = act.tile([1, 512], BF16, tag="grow", bufs=3)
                nc.gpsimd.dma_start(grow[0:1, :H * L], grep[:, wo * S:wo * S + L])
                rz = f32w.tile([1, 512], F32, tag="rz", bufs=1)
                nc.vector.reciprocal(rz[0:1, :H * L], psz[0:1, :H * L])
                wrow = f32w.tile([1, 512], F32R, tag="wrow", bufs=1)
                nc.vector.tensor_mul(
                    wrow[0:1, :H * L], rz[0:1, :H * L], grow[0:1, :H * L]
                )
                psw = psb.tile([P, 512], F32, tag="big")
                nc.tensor.matmul(
                    psw[:, :H * L],
                    lhsT=ones_row_f32[0:1, :],
                    rhs=wrow[0:1, :H * L],
                    start=True,
                    stop=True,
                )
                wbc = f32w.tile([P, 512], F32, tag="wbc", bufs=2)
                nc.scalar.copy(wbc[:, :H * L], psw[:, :H * L])
                for ft in range(FT):
                    psy = psb.tile([P, 512], F32, tag="big")
                    for h in range(H):
                        nc.tensor.matmul(
                            psy[:, h * L:h * L + L],
                            lhsT=xw[:L, w, ft * P:(ft + 1) * P],
                            rhs=atts[h][:L, :L],
                            start=True,
                            stop=True,
                        )
                    # scale by w_row and scatter into ysc[(h*FT+ft)]
                    nc.vector.tensor_mul(
                        ysc[:, :, wo * S:wo * S + L]
                        .rearrange("p (h f) m -> p h f m", f=FT)[:, :, ft, :],
                        psy[:, :H * L].rearrange("p (h m) -> p h m", m=L),
                        wbc[:, :H * L].rearrange("p (h m) -> p h m", m=L),
                    )
                wo += wn

            # ---- fused projection + residual ----
            x1u = f32w.tile([P, DPT, MC], F32R, tag="x1u")
            for dp in range(DPT):
                ps = psb.tile([P, 512], F32, tag="big")
                for t in range(TT):
                    nc.tensor.matmul(
                        ps[:, :MC],
                        lhsT=wvp_sb[:, t, dp * P:(dp + 1) * P],
                        rhs=ysc[:, t, :],
                        start=(t == 0),
                        stop=False,
                    )
                nc.tensor.matmul(
                    ps[:, :MC],
                    lhsT=cg_sb[:, dp * P:(dp + 1) * P],
                    rhs=grep[:, :],
                    start=False,
                    stop=True,
                )
                nc.vector.scalar_tensor_tensor(
                    out=x1u[:, dp, :],
                    in0=ps[:, :MC],
                    scalar=bpc_sb[:, dp:dp + 1],
                    in1=xt[:, dp, :],
                    op0=ALU.add,
                    op1=ALU.add,
                )

            # ---- layernorm helper (T-layout, stats via PE ones-matmul) ----
            def layernorm(xu, xn_dst, ln_row):
                sq = f32w.tile([P, DPT, MC], F32R, tag="sq")
                for dp in range(DPT):
                    nc.scalar.activation(sq[:, dp, :], xu[:, dp, :], AF.Square)
                pssum = psr.tile([1, 512], F32, tag="rows")
                pssq = psr.tile([1, 512], F32, tag="rows")
                for dp in range(DPT):
                    nc.tensor.matmul(
                        pssum[0:1, :MC],
                        lhsT=ones_col_f32[:, 0:1],
                        rhs=xu[:, dp, :],
                        start=(dp == 0),
                        stop=(dp == DPT - 1),
                    )
                    nc.tensor.matmul(
                        pssq[0:1, :MC],
                        lhsT=ones_col_f32[:, 0:1],
                        rhs=sq[:, dp, :],
                        start=(dp == 0),
                        stop=(dp == DPT - 1),
                    )
                mean = f32w.tile([1, 512], F32, tag="mean", bufs=1)
                nc.vector.tensor_scalar_mul(mean[0:1, :MC], pssum[0:1, :MC], 1.0 / D)
                var = f32w.tile([1, 512], F32, tag="var", bufs=1)
                # var = sumsq/D - mean^2
                nc.vector.tensor_mul(var[0:1, :MC], mean[0:1, :MC], mean[0:1, :MC])
                nc.vector.scalar_tensor_tensor(
                    out=var[0:1, :MC],
                    in0=pssq[0:1, :MC],
                    scalar=1.0 / D,
                    in1=var[0:1, :MC],
                    op0=ALU.mult,
                    op1=ALU.subtract,
                )
                std = f32w.tile([1, 512], F32, tag="std", bufs=1)
                nc.scalar.activation(std[0:1, :MC], var[0:1, :MC], AF.Sqrt, bias=eps_sb[0:1, 0:1])
                rstd = f32w.tile([1, 512], F32R, tag="rstd", bufs=1)
                nc.vector.reciprocal(rstd[0:1, :MC], std[0:1, :MC])
                # negmean_rstd = -mean * rstd
                nmr = f32w.tile([1, 512], F32R, tag="nmr", bufs=1)
                nc.vector.scalar_tensor_tensor(
                    out=nmr[0:1, :MC],
                    in0=mean[0:1, :MC],
                    scalar=-1.0,
                    in1=rstd[0:1, :MC],
                    op0=ALU.mult,
                    op1=ALU.mult,
                )
                psrs = psb.tile([P, 512], F32, tag="big")
                nc.tensor.matmul(
                    psrs[:, :MC],
                    lhsT=ones_row_f32[0:1, :],
                    rhs=rstd[0:1, :MC],
                    start=True,
                    stop=True,
                )
                psnm = psb.tile([P, 512], F32, tag="big")
                nc.tensor.matmul(
                    psnm[:, :MC],
                    lhsT=ones_row_f32[0:1, :],
                    rhs=nmr[0:1, :MC],
                    start=True,
                    stop=True,
                )
                for dp in range(DPT):
                    if apply_ln_affine:
                        tmp = f32w.tile([P, MC], F32, tag="lntmp", bufs=2)
                        nc.vector.scalar_tensor_tensor(
                            out=tmp[:, :],
                            in0=xu[:, dp, :],
                            scalar=1.0,
                            in1=psrs[:, :MC],
                            op0=ALU.mult,
                            op1=ALU.mult,
                        )
                        nc.vector.tensor_add(tmp[:, :], tmp[:, :], psnm[:, :MC])
                        nc.vector.tensor_scalar(
                            out=xn_dst(dp),
                            in0=tmp[:, :],
                            scalar1=ln_sb[:, ln_row, dp:dp + 1],
                            scalar2=ln_sb[:, ln_row + 1, dp:dp + 1],
                            op0=ALU.mult,
                            op1=ALU.add,
                        )
                    else:
                        tmp = f32w.tile([P, MC], F32, tag="lntmp", bufs=2)
                        nc.vector.tensor_mul(tmp[:, :], xu[:, dp, :], psrs[:, :MC])
                        nc.vector.tensor_add(xn_dst(dp), tmp[:, :], psnm[:, :MC])

            x1f = f32w.tile([P, DPT, MC], F32, tag="x1f")
            layernorm(x1u, lambda dp: x1f[:, dp, :], 0)
            x1n = act.tile([P, DPT, MC], BF16, tag="x1n", bufs=2)
            for dp in range(DPT):
                copy_out(evac_engine(dp), x1n[:, dp, :], x1f[:, dp, :])

            # ---- FFN (interleaved FFN1 -> relu -> FFN2 accumulation) ----
            pso = [psb.tile([P, 512], F32, tag="big", name=f"pso{_i}") for _i in range(DPT)]
            x2u = f32w.tile([P, DPT, MC], F32R, tag="x2u")
            for d1 in range(D1T):
                psf = psr.tile([P, 512], F32, tag="rows")
                for ft in range(FT):
                    nc.tensor.matmul(
                        psf[:, :MC],
                        lhsT=w1_sb[:, ft, d1 * P:(d1 + 1) * P],
                        rhs=x1n[:, ft, :],
                        start=(ft == 0),
                        stop=(ft == FT - 1),
                    )
                hrelu = act.tile([P, MC], BF16, tag="hrelu", bufs=6)
                if d1 % 2 == 0:
                    nc.vector.tensor_scalar(
                        out=hrelu[:, :],
                        in0=psf[:, :MC],
                        scalar1=b1c_sb[:, d1:d1 + 1],
                        scalar2=0.0,
                        op0=ALU.add,
                        op1=ALU.max,
                    )
                else:
                    nc.scalar.activation(
                        hrelu[:, :], psf[:, :MC], AF.Relu,
                        bias=b1c_sb[:, d1:d1 + 1],
                    )
                for dp in range(DPT):
                    nc.tensor.matmul(
                        pso[dp][:, :MC],
                        lhsT=w2_sb[:, d1, dp * P:(dp + 1) * P],
                        rhs=hrelu[:, :],
                        start=(d1 == 0),
                        stop=(d1 == D1T - 1),
                    )
            for dp in range(DPT):
                nc.vector.scalar_tensor_tensor(
                    out=x2u[:, dp, :],
                    in0=pso[dp][:, :MC],
                    scalar=b2c_sb[:, dp:dp + 1],
                    in1=x1f[:, dp, :],
                    op0=ALU.add,
                    op1=ALU.add,
                )

            x2n = f32w.tile([P, DPT, MC], F32, tag="x2n")
            layernorm(x2u, lambda dp: x2n[:, dp, :], 2)

            # ---- transpose back to row layout and store ----
            ccs = [(0, 128), (128, 128), (256, 64)]
            for cc, (c0, cw) in enumerate(ccs):
                osb = f32w.tile([P, F], F32, tag="osb", bufs=2)
                for dp in range(DPT):
                    pst = psr.tile([P, 512], F32, tag="rows")
                    nc.tensor.transpose(
                        pst[:cw, :P], x2n[:, dp, c0:c0 + cw], ident[:, :]
                    )
                    copy_out(
                        evac_engine(dp), osb[:cw, dp * P:(dp + 1) * P],
                        pst[:cw, :P],
                    )
                nc.sync.dma_start(
                    out_flat[m0 + c0:m0 + c0 + cw, :], osb[:cw, :]
                )

        _stack.close()

    nc.compile()
    return nc


def _prep_inputs(inputs):
    """Host-side weight fusion; returns per-core in_maps."""
    bf = ml_dtypes.bfloat16
    x = np.ascontiguousarray(inputs["x"], dtype=np.float32)
    Wq = inputs["Wq"].astype(np.float32)
    Wk = inputs["Wk"].astype(np.float32)
    Wv = inputs["Wv"].astype(np.float32)
    Wp = inputs["Wp"].astype(np.float32).reshape(H, D, D)
    sc = 1.0 / math.sqrt(D)
    wq_p = (Wq.transpose(1, 0, 2).reshape(F, H * D) * sc).astype(bf)
    wk_p = Wk.transpose(1, 0, 2).reshape(F, H * D).astype(bf)
    wvp_p = np.einsum("hfd,hde->hfe", Wv, Wp).reshape(H * F, D).astype(bf)
    cg_p = np.einsum("hd,hde->he", inputs["bv"].astype(np.float32), Wp).astype(bf)
    w1_p = inputs["W1"].astype(bf)
    w2_p = inputs["W2"].astype(bf)
    wg_p = (inputs["Wg"].astype(np.float32) / S).astype(bf)
    def col(v, nt):
        return np.ascontiguousarray(
            v.astype(np.float32).reshape(nt, 128).T
        )

    bqc_p = col(inputs["bq"].reshape(-1) * sc, TT)
    bkc_p = col(inputs["bk"].reshape(-1), TT)
    bpc_p = col(inputs["bp"], DPT)
    b1c_p = col(inputs["b1"], D1T)
    b2c_p = col(inputs["b2"], DPT)
    bg_p = inputs["bg"].astype(np.float32).reshape(1, H).astype(bf)
    ln_p = np.stack(
        [inputs["g1"], inputs["be1"], inputs["g2"], inputs["be2"]]
    ).astype(np.float32)
    apply_affine = not (
        np.all(ln_p[0] == 1) and np.all(ln_p[1] == 0)
        and np.all(ln_p[2] == 1) and np.all(ln_p[3] == 0)
    )
    shared = dict(
        wq_p=wq_p, wk_p=wk_p, wvp_p=wvp_p, cg_p=cg_p, w1_p=w1_p, w2_p=w2_p,
        wg_p=wg_p, bqc_p=bqc_p, bkc_p=bkc_p, bpc_p=bpc_p, b1c_p=b1c_p,
        b2c_p=b2c_p, bg_p=bg_p, mask_p=_make_mask(),
    )
    if apply_affine:
        shared["ln_p"] = ln_p
    x_bf = x.reshape(-1, F).astype(bf)
    in_maps = []
    for c in range(NCORES):
        m = dict(shared)
        m["x_bf"] = np.ascontiguousarray(x_bf[c * BC * S:(c + 1) * BC * S])
        in_maps.append(m)
    return in_maps, apply_affine


def _prep_inputs_small(inputs, nsamp):
    """Single map covering the first nsamp samples (for CoreSim tests)."""
    sub = dict(inputs)
    sub["x"] = np.asarray(inputs["x"])[:nsamp]
    maps, apply_affine = _prep_inputs(sub)
    m = maps[0]
    m["x_bf"] = m["x_bf"][: nsamp * S]
    return m, apply_affine


def _make_mask():
    m = np.zeros((120, 120), dtype=np.float32)
    for b in range(12):
        m[10 * b:10 * b + 10, 10 * b:10 * b + 10] = 1.0
    return m.astype(ml_dtypes.bfloat16)


_CACHED = {}


def _get_kernel(apply_affine):
    key = apply_affine
    if key not in _CACHED:
        _CACHED[key] = build_kernel(apply_affine)
    return _CACHED[key]


def kernel(**inputs):
    from concourse.bass_utils import run_bass_kernel_spmd

    in_maps, apply_affine = _prep_inputs(inputs)
    nc = _get_kernel(apply_affine)
    res = run_bass_kernel_spmd(nc, in_maps, list(range(NCORES)))
    outs = [np.asarray(r["out"]).reshape(BC, S, F) for r in res.results]
    return np.concatenate(outs, axis=0)


if __name__ == "__main__":
    nc = build_kernel(False)
    print("built ok")

